# revision 11
# baseline (speedup 1.0000x reference)
"""Multi-headed attention (B=2, S=2048, D=768, H=12) on 8 TRN2 NeuronCores.

Sharding: data parallel on batch x tensor parallel on heads. Core c handles
batch c//4 and heads 3*(c%4) .. 3*(c%4)+2. Each core computes its partial
output projection [S, D]; the host sums the 4 partials per batch.

Key-position compaction: the mask is per key position only ([B,1,1,S]).
The host drops masked key/value positions and pads to a multiple of 128.
The pad/mask handling lives entirely in v_aug: each (head, kblock) chunk of
v_aug carries 64 v columns plus a 0/1 "m01" column. The m01 column doubles
as the softmax-denominator accumulator (PV matmul row 64) AND as the mask:
pad positions have m01=0 and xv=0, so they contribute exactly nothing to
numerator or denominator. exp() therefore needs NO per-block mask bias and
runs on wide [128,1024] tiles with a constant 0 bias.

Everything is bf16 on the PE (1 cycle/row, no fp32r power throttle, half
the HBM traffic); accumulation stays f32 in PSUM. Softmax runs without
max-subtraction (exp(score) <= ~7000 fits fp32/bf16 fine).

Engine assignment: PE matmuls; ACT exp + the u->SBUF copy; DVE bias-adds,
v_aug copies, reciprocal, normalize-mul, out-proj PSUM->SBUF copies
(GpSimd has no PSUM port). The x-chunk SBUF pool is shared by xk/xv/xq so
pool WAR dependencies serialize the big DMA streams in compute order.
"""

import sys

for _p in ("/opt/trn_rl_repo",):
    if _p not in sys.path:
        sys.path.insert(0, _p)

import numpy as np

import concourse.bacc as bacc
import concourse.mybir as mybir
import concourse.tile as tile

B, S, D, H = 2, 2048, 768, 12
DK = D // H          # 64
NH = 3               # heads per core
E = NH * DK          # 192 local e width
N_CORES = 8
QN = 512             # q tile (moving free dim)
QC = S // QN         # 4
DCH = D // 128       # 6 contraction chunks for the projections

F32 = mybir.dt.float32
F32R = mybir.dt.float32r
BF16 = mybir.dt.bfloat16


def _build_program(kb: int):
    """Build the single-core SPMD program for KB key blocks of 128."""
    sk = kb * 128
    nc = bacc.Bacc("TRN2", target_bir_lowering=False, debug=False)

    xq = nc.dram_tensor("xq_t", [D, S], BF16, kind="ExternalInput").ap()
    xk = nc.dram_tensor("xk_t", [D, sk], BF16, kind="ExternalInput").ap()
    xv = nc.dram_tensor("xv_t", [D, sk], BF16, kind="ExternalInput").ap()
    wq = nc.dram_tensor("wq_t", [D, E], BF16, kind="ExternalInput").ap()
    wk = nc.dram_tensor("wk_t", [D, E], BF16, kind="ExternalInput").ap()
    wv = nc.dram_tensor("wv_t", [D, E], BF16, kind="ExternalInput").ap()
    wo = nc.dram_tensor("wo_t", [E, D], BF16, kind="ExternalInput").ap()
    bqk = nc.dram_tensor("bqk", [128, 4], F32, kind="ExternalInput").ap()
    m01 = nc.dram_tensor("m01t", [128, NH * kb], BF16, kind="ExternalInput").ap()
    ones_in = nc.dram_tensor("ones_in", [1, 512], F32R, kind="ExternalInput").ap()
    out = nc.dram_tensor("out", [S, D], BF16, kind="ExternalOutput").ap()

    exp_f = mybir.ActivationFunctionType.Exp

    # key-block groups of <=2 for wide [128, 1024] exp tiles
    groups = [tuple(range(t, min(t + 2, kb))) for t in range(0, kb, 2)]

    with tile.TileContext(nc) as tc:
        with (
            tc.tile_pool(name="resident", bufs=1) as res,
            tc.tile_pool(name="xin", bufs=6) as xin,
            tc.tile_pool(name="eT", bufs=3) as etp,
            tc.tile_pool(name="rec", bufs=2) as recp,
            tc.tile_pool(name="ucp", bufs=2) as ucp,
            tc.tile_pool(name="otp", bufs=2) as otp,
        ):
            # ---- resident SBUF ----
            qTp = res.tile([128, S], BF16, tag="qTp")     # heads 0,1
            qTs = res.tile([64, S], BF16, tag="qTs")      # head 2
            kTp = res.tile([128, sk], BF16, tag="kTp")
            kTs = res.tile([64, sk], BF16, tag="kTs")
            v_aug = res.tile([128, NH * kb * 65], BF16, tag="vaug")
            wo_sb = [
                res.tile([64, D], BF16, tag=f"wo{h}", name=f"wo{h}")
                for h in range(NH)
            ]
            bqk_sb = res.tile([128, 4], F32, tag="bqk")
            ones = res.tile([1, 512], F32R, tag="ones")
            wq_sb = res.tile([128, DCH * E], BF16, tag="wq")
            wk_sb = res.tile([128, DCH * E], BF16, tag="wk")
            wv_sb = res.tile([128, DCH * E], BF16, tag="wv")
            xT = [
                [
                    res.tile([64, QN], BF16, tag=f"xT{h}_{j}", name=f"xT{h}_{j}")
                    for j in range(QC)
                ]
                for h in range(NH)
            ]

            # ---- resident DMAs (emission order ~= queue order) ----
            nc.sync.dma_start(out=ones[:], in_=ones_in[0:1, :])
            for dc in range(DCH):
                nc.sync.dma_start(
                    out=wk_sb[:, dc * E:(dc + 1) * E], in_=wk[dc * 128:(dc + 1) * 128, :]
                )
            nc.sync.dma_start(out=bqk_sb[:], in_=bqk[:, :])
            # x chunks: shared pool forces xk -> xv -> xq stream order via WAR
            xk_ch = [xin.tile([128, S], BF16, tag="xch", name=f"xk{dc}") for dc in range(DCH)]
            for dc in range(DCH):
                nc.sync.dma_start(out=xk_ch[dc][:, :sk], in_=xk[dc * 128:(dc + 1) * 128, :])
            for dc in range(DCH):
                nc.sync.dma_start(
                    out=wv_sb[:, dc * E:(dc + 1) * E], in_=wv[dc * 128:(dc + 1) * 128, :]
                )
            xv_ch = [xin.tile([128, S], BF16, tag="xch", name=f"xv{dc}") for dc in range(DCH)]
            for dc in range(DCH):
                nc.sync.dma_start(out=xv_ch[dc][:, :sk], in_=xv[dc * 128:(dc + 1) * 128, :])
            nc.sync.dma_start(
                out=v_aug[:].rearrange("p (g c) -> p g c", c=65)[:, :, 64:65],
                in_=m01[:, :].rearrange("p (g o) -> p g o", o=1),
            )
            for dc in range(DCH):
                nc.sync.dma_start(
                    out=wq_sb[:, dc * E:(dc + 1) * E], in_=wq[dc * 128:(dc + 1) * 128, :]
                )
            xq_ch = [xin.tile([128, S], BF16, tag="xch", name=f"xq{dc}") for dc in range(DCH)]
            for dc in range(DCH):
                nc.sync.dma_start(out=xq_ch[dc][:], in_=xq[dc * 128:(dc + 1) * 128, :])
            for h in range(NH):
                nc.sync.dma_start(out=wo_sb[h][:], in_=wo[h * 64:(h + 1) * 64, :])

            # ---- phase P: projections ----
            with tc.tile_pool(name="proj_ps", bufs=4, space="PSUM") as proj_ps:
                # PE warm-up: fill the HAM activity window while xk streams in
                for _ in range(8):
                    ps = proj_ps.tile([128, QN], F32, tag="pp")
                    nc.tensor.matmul(
                        ps[:64, :], ones[0:1, 0:64], ones[0:1, :],
                        start=True, stop=True,
                    )

                # K then Q projections share structure
                def proj_qk(w_sb, xch, scols, pair, single, bcol_p, bcol_s):
                    for ec, ew, dst, bcol in (
                        (0, 128, pair, bcol_p),
                        (128, 64, single, bcol_s),
                    ):
                        for sc in range(0, scols, QN):
                            sw = min(QN, scols - sc)
                            ps = proj_ps.tile([128, QN], F32, tag="pp")
                            for dc in range(DCH):
                                nc.tensor.matmul(
                                    ps[:ew, :sw],
                                    w_sb[:, dc * E + ec:dc * E + ec + ew],
                                    xch[dc][:, sc:sc + sw],
                                    start=(dc == 0),
                                    stop=(dc == DCH - 1),
                                )
                            nc.vector.tensor_scalar_add(
                                dst[:ew, sc:sc + sw], ps[:ew, :sw],
                                bqk_sb[:ew, bcol:bcol + 1],
                            )

                proj_qk(wk_sb, xk_ch, sk, kTp, kTs, 1, 3)

                # V projection -> v_aug (natural [kpos, e] layout)
                v3 = v_aug[:].rearrange("p (h s c) -> p h s c", h=NH, c=65)
                for sb in range(kb):
                    ps = proj_ps.tile([128, QN], F32, tag="pp")
                    for dc in range(DCH):
                        nc.tensor.matmul(
                            ps[:, :E],
                            xv_ch[dc][:, sb * 128:(sb + 1) * 128],
                            wv_sb[:, dc * E:(dc + 1) * E],
                            start=(dc == 0),
                            stop=(dc == DCH - 1),
                        )
                    nc.vector.tensor_copy(
                        v3[:, :, sb, 0:64],
                        ps[:, 0:E].rearrange("p (h c) -> p h c", c=64),
                    )

                proj_qk(wq_sb, xq_ch, S, qTp, qTs, 0, 2)

            # ---- phase A+O: attention and output projection ----
            units = [(j, h) for j in range(QC) for h in range(NH)]
            pending = [None]

            with (
                tc.tile_pool(name="st_ps", bufs=2, space="PSUM") as st_ps,
                tc.tile_pool(name="u_ps", bufs=2, space="PSUM") as u_ps,
                tc.tile_pool(name="oa_ps", bufs=1, space="PSUM") as oa_ps,
                tc.tile_pool(name="ob_ps", bufs=1, space="PSUM") as ob_ps,
            ):
                def emit_out(j):
                    for qb in range(4 * j, 4 * j + 4):
                        cq = (qb % 4) * 128
                        ot = otp.tile([128, D], BF16, tag="ot")
                        for pool, f0, fw in ((oa_ps, 0, 512), (ob_ps, 512, 256)):
                            ps = pool.tile([128, fw], F32, tag="op")
                            for h in range(NH):
                                nc.tensor.matmul(
                                    ps[:, :],
                                    xT[h][qb // 4][:, cq:cq + 128],
                                    wo_sb[h][:, f0:f0 + fw],
                                    start=(h == 0),
                                    stop=(h == NH - 1),
                                )
                            if fw == 512:
                                nc.scalar.copy(ot[:, f0:f0 + fw], ps[:, :])
                            else:
                                nc.vector.tensor_copy(ot[:, f0:f0 + fw], ps[:, :])
                        nc.sync.dma_start(
                            out=out[qb * 128:(qb + 1) * 128, :], in_=ot[:, :]
                        )

                for i, (j, h) in enumerate(units):
                    if h < 2:
                        k_l = kTp[h * 64:(h + 1) * 64, :]
                        q_l = qTp[h * 64:(h + 1) * 64, :]
                    else:
                        k_l = kTs[:, :]
                        q_l = qTs[:, :]
                    u = u_ps.tile([65, QN], F32, tag="u")

                    prev = None
                    for gi, grp in enumerate(groups):
                        gw = len(grp) * QN
                        st = st_ps.tile([128, 2 * QN], F32, tag="st")
                        for bi, b_ in enumerate(grp):
                            nc.tensor.matmul(
                                st[:, bi * QN:(bi + 1) * QN],
                                k_l[:, b_ * 128:(b_ + 1) * 128],
                                q_l[:, j * QN:(j + 1) * QN],
                                start=True,
                                stop=True,
                            )
                        et = etp.tile([128, 2 * QN], BF16, tag="et")
                        nc.scalar.activation(
                            et[:, :gw], st[:, :gw], exp_f, scale=0.125,
                        )
                        if gi == 0 and pending[0] is not None:
                            pending[0]()
                            pending[0] = None
                        if prev is not None:
                            pgrp, pet = prev
                            for bi, b_ in enumerate(pgrp):
                                nc.tensor.matmul(
                                    u[:, :],
                                    v_aug[:, (h * kb + b_) * 65:(h * kb + b_) * 65 + 65],
                                    pet[:, bi * QN:(bi + 1) * QN],
                                    start=(b_ == 0),
                                    stop=(b_ == kb - 1),
                                )
                        prev = (grp, et)
                    pgrp, pet = prev
                    for bi, b_ in enumerate(pgrp):
                        nc.tensor.matmul(
                            u[:, :],
                            v_aug[:, (h * kb + b_) * 65:(h * kb + b_) * 65 + 65],
                            pet[:, bi * QN:(bi + 1) * QN],
                            start=(b_ == 0),
                            stop=(b_ == kb - 1),
                        )

                    rec = recp.tile([1, QN], F32, tag="rec")
                    nc.vector.reciprocal(rec[:, :], u[64:65, :])
                    bcast = ucp.tile([64, QN], F32, tag="bcast")
                    nc.gpsimd.partition_broadcast(bcast[:, :], rec[0:1, :])

                    def mk_pending(u=u, bcast=bcast, j=j, h=h):
                        def emit():
                            nc.vector.tensor_mul(
                                xT[h][j][:, :], u[0:64, :], bcast[:, :]
                            )
                        return emit

                    pending[0] = mk_pending()

                    # out proj of row-block j after attention unit (j+1, 0)
                    if h == 0 and j > 0:
                        emit_out(j - 1)

                pending[0]()
                pending[0] = None
                emit_out(QC - 1)

    nc.compile()
    return nc


_PROGRAM_CACHE: dict[int, object] = {}


def _get_program(kb: int):
    if kb not in _PROGRAM_CACHE:
        _PROGRAM_CACHE[kb] = _build_program(kb)
    return _PROGRAM_CACHE[kb]


def _prep_inputs(query, key, value, mask, Wq, bq, Wk, bk, Wv, bv, Wo, bo):
    """Host-side shard prep. Returns (in_maps, kb)."""
    import ml_dtypes

    bf = ml_dtypes.bfloat16
    f32 = np.float32
    valid = [np.nonzero(mask[b, 0, 0, :] != 0)[0] for b in range(B)]
    s_valid = max((len(v) for v in valid), default=1)
    s_pad = max(128, -(-s_valid // 128) * 128)
    kb = s_pad // 128

    per_batch = []
    for b in range(B):
        vi = valid[b]
        xk_c = np.zeros((s_pad, D), dtype=f32)
        xv_c = np.zeros((s_pad, D), dtype=f32)
        xk_c[: len(vi)] = key[b][vi]
        xv_c[: len(vi)] = value[b][vi]
        m01_pb = np.zeros((s_pad,), dtype=f32)
        m01_pb[: len(vi)] = 1.0
        m01_pb = np.ascontiguousarray(m01_pb.reshape(kb, 128).T)  # [128, kb]
        per_batch.append(
            dict(
                xq_t=np.ascontiguousarray(query[b].T).astype(bf),
                xk_t=np.ascontiguousarray(xk_c.T).astype(bf),
                xv_t=np.ascontiguousarray(xv_c.T).astype(bf),
                m01t=np.ascontiguousarray(np.tile(m01_pb, (1, NH))).astype(bf),
            )
        )

    in_maps = []
    for c in range(N_CORES):
        b = c // 4
        h0 = NH * (c % 4)
        sl = slice(h0 * DK, (h0 + NH) * DK)
        bqk_ = np.zeros((128, 4), dtype=f32)
        bqk_[:, 0] = bq[sl][:128]
        bqk_[:, 1] = bk[sl][:128]
        bqk_[:64, 2] = bq[sl][128:]
        bqk_[:64, 3] = bk[sl][128:]
        in_maps.append(
            dict(
                per_batch[b],
                wq_t=np.ascontiguousarray(Wq[sl, :].T).astype(bf),
                wk_t=np.ascontiguousarray(Wk[sl, :].T).astype(bf),
                wv_t=np.ascontiguousarray(Wv[sl, :].T).astype(bf),
                wo_t=np.ascontiguousarray(Wo[:, sl].T).astype(bf),
                bqk=bqk_,
                ones_in=np.ones((1, 512), dtype=f32),
            )
        )
    return in_maps, kb


def kernel(query, key, value, mask, Wq, bq, Wk, bk, Wv, bv, Wo, bo):
    from concourse.bass_utils import run_bass_kernel_spmd

    query = np.asarray(query, dtype=np.float32)
    key = np.asarray(key, dtype=np.float32)
    value = np.asarray(value, dtype=np.float32)
    mask = np.asarray(mask)
    Wq, Wk, Wv, Wo = (np.asarray(a, dtype=np.float32) for a in (Wq, Wk, Wv, Wo))
    bq, bk, bv, bo = (np.asarray(a, dtype=np.float32) for a in (bq, bk, bv, bo))

    in_maps, kb = _prep_inputs(
        query, key, value, mask, Wq, bq, Wk, bk, Wv, bv, Wo, bo
    )
    nc = _get_program(kb)
    res = run_bass_kernel_spmd(nc, in_maps, core_ids=list(range(N_CORES)))

    out = np.zeros((B, S, D), dtype=np.float32)
    for c in range(N_CORES):
        out[c // 4] += res.results[c]["out"].astype(np.float32)
    # bv folds into the output as (sum_k p == 1) -> + bv @ Wo.T; bo is a plain
    # output bias. Both are zero for this problem's inputs; keep exactness for
    # any input without on-device cost.
    if np.any(bv) or np.any(bo):
        out += (bv @ Wo.T + bo)[None, None, :]
    return out
